# revision 3
# baseline (speedup 1.0000x reference)
"""Trainium2 Bass kernel for CustomMultiheadAttention (linear attention with
low-rank QKV projections) — sequence-sharded, bf16, v5.

Math (fp32 reference):
    q = elu(query @ Wq.T + q_up_b) + 1      with Wq = q_up_w @ q_down_w  [E,E]
    k = elu(key   @ Wk.T + k_up_b) + 1
    v =      value @ Wv.T (+ v_up_b: folded into the output bias on host,
             since attention is an affine average: attn(v + b) = attn(v) + b
             up to the 1e-6 denominator fudge, an O(1e-11) difference)
    per head h (16 heads, head_dim 64):
        kv_h   = k_h^T v_h      (sum over ALL tokens)
        ksum_h = sum_t k_h[t]
        attn_h = (q_h kv_h) / (q_h . ksum_h + 1e-6)
    out = concat_h(attn_h) @ out_w.T + out_b + v_up_b @ out_w.T

Sharding: 8 cores = 4 batches x 2 sequence-halves (2048 tokens per core) —
an exact 1/8 split of every per-token matmul. Cross-core data is only the
per-head kv/ksum accumulators: a 2-core AllReduce ({0,1},{2,3},{4,5},{6,7})
of a packed 132 KB bf16 payload, overlapped with the q-projection matmuls.

All matmul operands are bf16 (PSUM accumulation stays fp32; verified
max-rel-err ~8e-4 vs the fp32 reference on the host). bf16 halves DMA and
SBUF and enables fast weight load; the PE streaming rate is the same
1 cycle/row as f32r, so the matmul schedule/shape matches the f32r version.

Feature map: elu(u)+1 = min(exp(u),1) + relu(u), two scalar-engine ACTs
(bias folded in on the q side where it is per-partition) + one DVE op.

ksum is fused into the kv matmul: the v tile carries two extra all-ones
columns per j-tile, so each kv matmul (moving dim 130) also produces the
k-feature column sums.
"""

import numpy as np
from ml_dtypes import bfloat16

import concourse.bass as bass  # noqa: F401
import concourse.mybir as mybir
import concourse.tile as tile
from concourse import bacc
from concourse.bass_utils import run_bass_kernel_spmd

F32 = mybir.dt.float32
BF16 = mybir.dt.bfloat16
AF = mybir.ActivationFunctionType
OP = mybir.AluOpType

P = 128           # partitions
E = 1024          # embed dim
NH = 16           # heads
S_LOC = 2048      # tokens per core
TC = 512          # token chunk
NCH = S_LOC // TC  # 4 chunks
NE = E // P       # 8 e-tiles
NJ = E // P       # 8 j-tiles (2 heads each)
NTS = TC // P     # 4 token subtiles per chunk
KW = P + 2        # kv block width: 128 v-cols + 2 ones-cols (ksum)
AW = 66           # packed allreduce width/jt: two 64x64 diag blocks + ksum

_CACHE = {}


def _build():
    nc = bacc.Bacc(None, target_bir_lowering=False, num_devices=8)

    # all inputs are pre-tiled on the host into device layout so every DMA
    # is one contiguous run per partition (the DMA engines are descriptor-
    # rate-bound at startup; 1 KB strided descriptors cost ~20 us)
    dp = nc.declare_dram_parameter
    xq = dp("xq", [NCH, P, NE, TC], BF16, isOutput=False)
    xk = dp("xk", [NCH, P, NE, TC], BF16, isOutput=False)
    xv = dp("xv", [NCH, P, NE, TC], BF16, isOutput=False)
    wq = dp("wq", [P, NE, E], BF16, isOutput=False)  # (q_up@q_down).T tiled
    wk = dp("wk", [2, P, NE, TC], BF16, isOutput=False)  # j-halves
    wv = dp("wv", [2, P, NE, TC], BF16, isOutput=False)
    wo = dp("wo", [P, NJ, E], BF16, isOutput=False)  # out_w.T tiled
    cst = dp("cst", [P, NJ * NH + NJ + E], F32, isOutput=False)  # rtm|bqt|bkb
    r8m = dp("r8m", [NH, E], BF16, isOutput=False)      # head-replication mask
    out_t = dp("out", [S_LOC, E], F32, isOutput=True)

    with tile.TileContext(nc) as tcx:
        from contextlib import ExitStack

        with ExitStack() as root:
            cpool = root.enter_context(tcx.tile_pool(name="consts", bufs=1))
            xqpool = root.enter_context(tcx.tile_pool(name="xqp", bufs=4))
            dram = root.enter_context(
                tcx.tile_pool(name="dram", bufs=1, space="DRAM"))
            csb = cpool.tile([P, NJ * NH + NJ + E], F32)
            rt_sb = csb[:, 0:NJ * NH]
            bqt_sb = csb[:, NJ * NH:NJ * NH + NJ]
            bkb_sb = csb[:, NJ * NH + NJ:]
            r8_sb = cpool.tile([NH, E], BF16)
            abuf = cpool.tile([P, NJ, AW], BF16)   # packed allreduce payload
            kv2r = cpool.tile([P, NJ, P], BF16)    # reduced block-diag kv
            ksr = cpool.tile([P, NJ], F32)         # reduced ksum
            kblk = cpool.tile([P, NJ * NH], BF16)  # ksum masked into head cols
            wq_sb = cpool.tile([P, NE, E], BF16)   # prefetched during phase KV
            ab_in = dram.tile([P, NJ, AW], BF16)
            ab_out = dram.tile([P, NJ, AW], BF16)

            nc.sync.dma_start(out=csb[:], in_=cst[:])
            nc.sync.dma_start(out=r8_sb[:], in_=r8m[:])
            # kv2r is built from the packed payload; off-diagonal stays zero
            nc.vector.memset(kv2r[:].bitcast(mybir.dt.uint16), 0.0)

            # ---------------- Phase KV ----------------
            with ExitStack() as ph:
                wpool = ph.enter_context(tcx.tile_pool(name="wkv", bufs=1))
                wk_h = [wpool.tile([P, NE, TC], BF16, name=f"wk{i}") for i in range(2)]
                wv_h = [wpool.tile([P, NE, TC], BF16, name=f"wv{i}") for i in range(2)]
                # weights ride the scalar/gpsimd DMA queues so the x chunks
                # (sync queue) land in parallel; wk j-half 0 lands first so
                # the first feature matmul starts after ~1 MB of DMA
                nc.scalar.dma_start(out=wk_h[0][:], in_=wk[0])
                nc.scalar.dma_start(out=wk_h[1][:], in_=wk[1])
                nc.gpsimd.dma_start(wv_h[0][:], wv[0])
                nc.gpsimd.dma_start(wv_h[1][:], wv[1])
                nc.scalar.dma_start(out=wq_sb[:], in_=wq[:])

                xkpool = ph.enter_context(tcx.tile_pool(name="xkp", bufs=2))
                xvpool = ph.enter_context(tcx.tile_pool(name="xvp", bufs=2))
                xk0t = xkpool.tile([P, NE, TC], BF16, tag="xk", name="xk")
                xv0t = xvpool.tile([P, NE, TC], BF16, tag="xv", name="xv")
                nc.sync.dma_start(out=xk0t[:], in_=xk[0])
                nc.sync.dma_start(out=xv0t[:], in_=xv[0])
                fpool = ph.enter_context(tcx.tile_pool(name="fkv", bufs=1))
                tpool = ph.enter_context(tcx.tile_pool(name="tkv", bufs=1))
                psf = ph.enter_context(
                    tcx.tile_pool(name="psf", bufs=2, space="PSUM"))
                psk = ph.enter_context(
                    tcx.tile_pool(name="psk", bufs=1, space="PSUM"))

                kfeat = fpool.tile([P, NTS, E], BF16)
                vch = fpool.tile([P, NTS, NJ, KW], BF16)
                kvacc = fpool.tile([P, NJ, KW], F32)
                # ones columns for the fused ksum (bf16 1.0 = 0x3F80)
                nc.vector.memset(vch[:].bitcast(mybir.dt.uint16), 0x3F80)

                for ci in range(NCH):
                    if ci == 0:
                        xkt, xvt = xk0t, xv0t
                    else:
                        xkt = xkpool.tile([P, NE, TC], BF16, tag="xk",
                                          name="xk")
                        xvt = xvpool.tile([P, NE, TC], BF16, tag="xv",
                                          name="xv")
                        nc.sync.dma_start(out=xkt[:], in_=xk[ci])
                        nc.sync.dma_start(out=xvt[:], in_=xv[ci])

                    # k features: elu(x @ Wk.T + b) + 1, token-major [t, j]
                    for tb in range(NTS):
                        for jh in range(2):
                            pu = psf.tile([P, TC], F32, tag=f"ph{jh}",
                                          name="pu")
                            for e in range(NE):
                                nc.tensor.matmul(
                                    pu[:],
                                    xkt[:, e, P * tb:P * (tb + 1)],
                                    wk_h[jh][:, e, :],
                                    start=(e == 0), stop=(e == NE - 1),
                                )
                            js = slice(TC * jh, TC * (jh + 1))
                            u = tpool.tile([P, TC], F32, tag="u", name="u")
                            r = tpool.tile([P, TC], F32, tag="r", name="r")
                            ex = tpool.tile([P, TC], F32, tag="ex", name="ex")
                            nc.vector.tensor_tensor(
                                u[:], pu[:], bkb_sb[:, js], op=OP.add)
                            nc.scalar.activation(r[:], u[:], AF.Relu)
                            nc.scalar.activation(ex[:], u[:], AF.Exp)
                            nc.vector.scalar_tensor_tensor(
                                kfeat[:, tb, js], ex[:], 1.0, r[:],
                                op0=OP.min, op1=OP.add,
                            )

                    # v features: x @ Wv.T (bias folded into out bias)
                    for tb in range(NTS):
                        for jh in range(2):
                            pu = psf.tile([P, TC], F32, tag=f"ph{jh}",
                                          name="pu")
                            for e in range(NE):
                                nc.tensor.matmul(
                                    pu[:],
                                    xvt[:, e, P * tb:P * (tb + 1)],
                                    wv_h[jh][:, e, :],
                                    start=(e == 0), stop=(e == NE - 1),
                                )
                            nc.vector.tensor_copy(
                                vch[:, tb, 4 * jh:4 * (jh + 1), 0:P], pu[:])

                    # kv[j1, (j2|ones)] += sum_t kfeat[t, j1] vch[t, j1-tile]
                    pkv = [
                        psk.tile([P, 3 * KW], F32, tag="pkv0", name="pkv0"),
                        psk.tile([P, 3 * KW], F32, tag="pkv1", name="pkv1"),
                        psk.tile([P, 2 * KW], F32, tag="pkv2", name="pkv2"),
                    ]
                    for jt in range(NJ):
                        dst = pkv[jt // 3][:, KW * (jt % 3):KW * (jt % 3 + 1)]
                        jb = slice(P * jt, P * (jt + 1))
                        for ts in range(NTS):
                            nc.tensor.matmul(
                                dst, kfeat[:, ts, jb], vch[:, ts, jt, :],
                                start=(ts == 0), stop=(ts == NTS - 1),
                            )
                    if ci == 0:
                        nc.vector.tensor_copy(kvacc[:, 0:3, :], pkv[0][:])
                        nc.vector.tensor_copy(kvacc[:, 3:6, :], pkv[1][:])
                        nc.vector.tensor_copy(kvacc[:, 6:8, :], pkv[2][:])
                    else:
                        nc.vector.tensor_tensor(
                            kvacc[:, 0:3, :], kvacc[:, 0:3, :], pkv[0][:],
                            op=OP.add)
                        nc.vector.tensor_tensor(
                            kvacc[:, 3:6, :], kvacc[:, 3:6, :], pkv[1][:],
                            op=OP.add)
                        nc.vector.tensor_tensor(
                            kvacc[:, 6:8, :], kvacc[:, 6:8, :], pkv[2][:],
                            op=OP.add)

                # prefetch the first two q chunks (sync queue, after the
                # kv-side x chunks)
                xqts = []
                for qi in range(NCH):
                    xqt_p = xqpool.tile([P, NE, TC], BF16, tag="xq",
                                        name="xqt_p")
                    nc.sync.dma_start(out=xqt_p[:], in_=xq[qi])
                    xqts.append(xqt_p)

                # pack the allreduce payload: per j-tile the two 64x64
                # diagonal head blocks stacked plus the ksum columns
                for jt in range(NJ):
                    nc.vector.tensor_copy(
                        abuf[0:64, jt, 0:64], kvacc[0:64, jt, 0:64])
                    nc.vector.tensor_copy(
                        abuf[64:P, jt, 0:64], kvacc[64:P, jt, 64:P])
                    nc.vector.tensor_copy(
                        abuf[:, jt, 64:AW], kvacc[:, jt, P:P + 2])

            # pairwise AllReduce of (kv2, ksum); overlaps phase-Q matmuls
            nc.gpsimd.dma_start(ab_in[:], abuf[:])
            nc.gpsimd.collective_compute(
                "AllReduce",
                OP.add,
                replica_groups=[[0, 1], [2, 3], [4, 5], [6, 7]],
                ins=[ab_in[:].opt()],
                outs=[ab_out[:].opt()],
            )
            nc.gpsimd.dma_start(abuf[:], ab_out[:])
            # everything downstream of the AllReduce is emitted under a late
            # scheduling timestamp: the Tile scheduler's cost model treats
            # the collective as near-instant, so without this it interleaves
            # AR-dependent ops into the scalar/vector queues between Q1
            # chunks, and the in-order engine queues then head-of-line block
            # on the AR for tens of us
            with tcx.tile_wait_until(0.5):
                # unpack into the block-diagonal stationary + ksum
                for jt in range(NJ):
                    nc.vector.tensor_copy(
                        kv2r[0:64, jt, 0:64], abuf[0:64, jt, 0:64])
                    nc.vector.tensor_copy(
                        kv2r[64:P, jt, 64:P], abuf[64:P, jt, 0:64])
                nc.vector.tensor_copy(ksr[:], abuf[:, :, 64:65])

                # kblk[p, jt, h] = ksum[dim] if dim in head h else 0
                for jt in range(NJ):
                    hs = slice(NH * jt, NH * (jt + 1))
                    nc.vector.tensor_scalar(
                        kblk[:, hs], rt_sb[:, hs],
                        ksr[:, jt:jt + 1], None, op0=OP.mult)

            # ---------------- Phase Q + attention + output ----------------
            with ExitStack() as ph:
                wpool = ph.enter_context(tcx.tile_pool(name="wqo", bufs=1))
                wo_sb = wpool.tile([P, NJ, E], BF16)
                nc.scalar.dma_start(out=wo_sb[:], in_=wo[:])

                qpool = ph.enter_context(tcx.tile_pool(name="qf", bufs=4))
                tpool = ph.enter_context(tcx.tile_pool(name="tq", bufs=2))
                apool = ph.enter_context(tcx.tile_pool(name="attn", bufs=2))
                rpool = ph.enter_context(tcx.tile_pool(name="rep", bufs=2))
                opool = ph.enter_context(tcx.tile_pool(name="osb", bufs=2))
                psq = ph.enter_context(
                    tcx.tile_pool(name="psq", bufs=2, space="PSUM"))
                psd = ph.enter_context(
                    tcx.tile_pool(name="psd", bufs=1, space="PSUM"))
                psn = ph.enter_context(
                    tcx.tile_pool(name="psn", bufs=1, space="PSUM"))
                psr = ph.enter_context(
                    tcx.tile_pool(name="psr", bufs=1, space="PSUM"))
                pso = ph.enter_context(
                    tcx.tile_pool(name="pso", bufs=1, space="PSUM"))

                def emit_q(ci, xqt=None):
                    """q features for chunk ci, [j-part, t] layout."""
                    if xqt is None:
                        xqt = xqpool.tile([P, NE, TC], BF16, tag="xq",
                                          name="xq")
                        nc.sync.dma_start(out=xqt[:], in_=xq[ci])
                    qT = qpool.tile([P, NJ, TC], BF16, tag="qT", name="qT")
                    for jt in range(NJ):
                        pq = psq.tile([P, TC], F32, tag="pq", name="pq")
                        for e in range(NE):
                            nc.tensor.matmul(
                                pq[:], wq_sb[:, e, P * jt:P * (jt + 1)],
                                xqt[:, e, :],
                                start=(e == 0), stop=(e == NE - 1),
                            )
                        bq_ap = bqt_sb[:, jt:jt + 1]
                        r = tpool.tile([P, TC], F32, tag="qr", name="qr")
                        ex = tpool.tile([P, TC], F32, tag="qe", name="qe")
                        nc.scalar.activation(r[:], pq[:], AF.Relu, bias=bq_ap)
                        nc.scalar.activation(ex[:], pq[:], AF.Exp, bias=bq_ap)
                        nc.vector.scalar_tensor_tensor(
                            qT[:, jt, :], ex[:], 1.0, r[:],
                            op0=OP.min, op1=OP.add,
                        )
                    return qT

                def emit_den(ci, qT):
                    """denominator reciprocal for chunk ci (runs early so the
                    slow [16, TC] reciprocal hides under other PE work)."""
                    pdn = psd.tile([NH, TC], F32, tag="pdn", name="pdn")
                    for jt in range(NJ):
                        nc.tensor.matmul(
                            pdn[:], kblk[:, NH * jt:NH * (jt + 1)],
                            qT[:, jt, :],
                            start=(jt == 0), stop=(jt == NJ - 1),
                        )
                    dpl = tpool.tile([NH, TC], F32, tag="dpl", name="dpl")
                    rcp = tpool.tile([NH, TC], BF16, tag="rcp", name="rcp")
                    nc.vector.tensor_scalar_add(dpl[:], pdn[:], 1e-6)
                    with nc.allow_low_precision(
                            reason="bf16 reciprocal feeds the rep matmul; "
                                   "~0.4% is far inside the 2e-2 gate"):
                        nc.vector.reciprocal(rcp[:], dpl[:])
                    return rcp

                def emit_attn_out(ci, qT, rcp):
                    attn = apool.tile([P, NJ, TC], BF16, tag="attn",
                                      name="attn")
                    for jt in range(NJ):
                        pnm = psn.tile([P, TC], F32, tag=f"pnm{jt % 2}",
                                       name="pnm")
                        nc.tensor.matmul(
                            pnm[:], kv2r[:, jt, :], qT[:, jt, :],
                            start=True, stop=True,
                        )
                        prp = psr.tile([P, TC], F32, tag="prp", name="prp")
                        nc.tensor.matmul(
                            prp[:], r8_sb[:, P * jt:P * (jt + 1)], rcp[:],
                            start=True, stop=True,
                        )
                        rep = rpool.tile([P, TC], F32, tag="rep", name="rep")
                        nc.scalar.copy(rep[:], prp[:])
                        nc.vector.tensor_tensor(
                            attn[:, jt, :], pnm[:], rep[:], op=OP.mult)

                    # out[t, o] = sum_j attn[j, t] wo[j, o]; the two o-halves
                    # share each stationary attn block
                    for tb in range(NTS):
                        ob = opool.tile([P, 2, TC], F32, tag="ob", name="ob")
                        po = [pso.tile([P, TC], F32, tag=f"po{oh}",
                                       name="po") for oh in range(2)]
                        for jt in range(NJ):
                            for oh in range(2):
                                nc.tensor.matmul(
                                    po[oh][:],
                                    attn[:, jt, P * tb:P * (tb + 1)],
                                    wo_sb[:, jt, TC * oh:TC * (oh + 1)],
                                    start=(jt == 0), stop=(jt == NJ - 1),
                                )
                        nc.scalar.copy(ob[:, 0, :], po[0][:])
                        nc.vector.tensor_copy(ob[:, 1, :], po[1][:])
                        row0 = ci * TC + tb * P
                        nc.sync.dma_start(
                            out=out_t[row0:row0 + P, :].rearrange(
                                "p (a b) -> p a b", a=2),
                            in_=ob[:],
                        )

                # run q-projection chunks ahead so the kv AllReduce hides
                # under them; denominators early so reciprocals hide too.
                # NOTE: keep this exact interleaving — emitting all four
                # emit_q before the first emit_den makes the scheduler hoist
                # AR-dependent matmuls ahead of Q1 work in the in-order PE
                # queue (head-of-line blocking, ~100us)
                qTs = [emit_q(ci, xqts[ci]) for ci in range(NCH)]
                with tcx.tile_wait_until(0.5):
                    rcp0 = emit_den(0, qTs[0])
                    emit_attn_out(0, qTs[0], rcp0)
                    rcp1 = emit_den(1, qTs[1])
                    emit_attn_out(1, qTs[1], rcp1)
                    rcp2 = emit_den(2, qTs[2])
                    emit_attn_out(2, qTs[2], rcp2)
                    rcp3 = emit_den(3, qTs[3])
                    emit_attn_out(3, qTs[3], rcp3)

    nc.compile()
    return nc


def _get_nc():
    if "nc" not in _CACHE:
        _CACHE["nc"] = _build()
    return _CACHE["nc"]


def kernel(**inputs):
    query = np.asarray(inputs["query"], dtype=np.float32)
    key = np.asarray(inputs["key"], dtype=np.float32)
    value = np.asarray(inputs["value"], dtype=np.float32)

    f32 = np.float32
    Wq = (inputs["q_up_w"] @ inputs["q_down_w"]).astype(f32)   # [E, E]
    Wk = (inputs["k_up_w"] @ inputs["k_down_w"]).astype(f32)
    Wv = (inputs["v_up_w"] @ inputs["v_down_w"]).astype(f32)
    def tile_in(wT):
        # [E_in, N] -> [P, NE, N] with in-index = a*P + p
        return np.ascontiguousarray(
            wT.reshape(NE, P, -1).transpose(1, 0, 2)).astype(bfloat16)

    def tile_half(wT):
        # [E_in, E_out] -> [2, P, NE, TC] split on the out j-halves
        return np.ascontiguousarray(
            wT.reshape(NE, P, 2, TC).transpose(2, 1, 0, 3)).astype(bfloat16)

    com = {
        "wq": tile_in(Wq.T),
        "wk": tile_half(Wk.T),
        "wv": tile_half(Wv.T),
        "wo": tile_in(np.asarray(inputs["out_w"], f32).T),
    }
    # head masks: full[d, h] = 1 iff dim d belongs to head h
    heads = np.arange(E) // 64
    full = (heads[:, None] == np.arange(NH)[None, :]).astype(f32)
    rtm = full.reshape(NJ, P, NH).transpose(1, 0, 2).reshape(P, NJ * NH)
    bqt = np.asarray(inputs["q_up_b"], f32).reshape(NJ, P).T
    bkb = np.broadcast_to(np.asarray(inputs["k_up_b"], f32), (P, E))
    com["cst"] = np.ascontiguousarray(
        np.concatenate([rtm, bqt, bkb], axis=1), dtype=f32)
    com["r8m"] = np.ascontiguousarray(full.T).astype(bfloat16)  # [16, E]

    in_maps = []
    for c in range(8):
        b, h = divmod(c, 2)
        ts = slice(h * S_LOC, (h + 1) * S_LOC)
        def tile_x(x):
            # [S_LOC, E] -> [NCH, P, NE, TC]: chunk ci, partition p, e-tile a
            # holds x.T[a*P + p, ci*TC + t]
            return np.ascontiguousarray(
                x.T.reshape(NE, P, NCH, TC).transpose(2, 1, 0, 3)
            ).astype(bfloat16)

        im = {
            "xq": tile_x(query[b, ts]),
            "xk": tile_x(key[b, ts]),
            "xv": tile_x(value[b, ts]),
        }
        im.update(com)
        in_maps.append(im)

    nc = _get_nc()
    # the first execution after a device wedge occasionally dies with
    # NRT_EXEC_UNIT_UNRECOVERABLE; a retry on a clean session recovers
    last_err = None
    for _attempt in range(3):
        try:
            res = run_bass_kernel_spmd(nc, in_maps, core_ids=list(range(8)),
                                       **_CACHE.get("run_kwargs", {}))
            last_err = None
            break
        except Exception as e:  # noqa: BLE001
            last_err = e
            import time
            time.sleep(10)
    if last_err is not None:
        raise last_err
    _CACHE["last_result"] = res

    # v bias passes through the attention average; fold it into the out bias
    out_b = np.asarray(inputs["out_b"], f32) + (
        np.asarray(inputs["out_w"], f32) @ np.asarray(inputs["v_up_b"], f32))
    B = query.shape[0]
    out = np.empty((B, 2 * S_LOC, E), np.float32)
    for c in range(8):
        b, h = divmod(c, 2)
        out[b, h * S_LOC:(h + 1) * S_LOC] = res.results[c]["out"] + out_b
    return out
